# revision 1
# baseline (speedup 1.0000x reference)
"""Trainium2 Bass kernel for per-pixel kernel application (KPN-style ApplyKernel).

y[c,h,w] = sum_{ii,jj} xpad[c, h+ii, w+jj] * k[ii*11+jj, h, w]

Strategy (8 NeuronCores, fully data-parallel over W strips of 160 cols):
  - Host: pad x, build per-partition row-window slabs (6 rows + 10 halo per
    partition, in the free dim) in bf16, two column-alignment variants so every
    tap's DVE read stays 4-byte aligned (keeps tensor_tensor in 2x bf16 mode).
  - Device per tap: HWDGE DMA k plane (f32) -> ScalarE cast to bf16 ->
    VectorE tensor_tensor multiply (bf16, 2x mode) -> TensorE identity-matmul
    accumulates products into 6 PSUM banks (3 channels x 2 col-chunks).
  - Epilogue: ScalarE evacuates PSUM -> SBUF, one DMA out per core.
"""

import sys

if "/opt/trn_rl_repo" not in sys.path:
    sys.path.insert(0, "/opt/trn_rl_repo")

import numpy as np
import ml_dtypes

import concourse.mybir as mybir
from concourse import bacc
from concourse.tile import TileContext
from concourse.bass_utils import run_bass_kernel_spmd

KS = 11
HALF = 5
H, W, C = 720, 1280, 3
NCORES = 8
WS = W // NCORES            # 160 cols per core
RPP = 6                     # output rows per partition
NP = H // RPP               # 120 partitions used
ROWS_PP = RPP + 2 * HALF    # 16 rows stored per partition
WPAD = WS + 2 * HALF        # 170 cols stored per partition
SLABF = C * ROWS_PP * WPAD  # 8160 bf16 per partition per variant
NTAPS = KS * KS             # 121

BF16 = ml_dtypes.bfloat16

_CACHE = {}


def _build_nc(taps=NTAPS):
    nc = bacc.Bacc("TRN2", target_bir_lowering=False, debug=False)
    k_d = nc.dram_tensor("k", [NTAPS, H, WS], mybir.dt.float32, kind="ExternalInput")
    xs_d = nc.dram_tensor("xs", [2, 128, SLABF], mybir.dt.bfloat16, kind="ExternalInput")
    id_d = nc.dram_tensor("ident", [NP, NP], mybir.dt.bfloat16, kind="ExternalInput")
    y_d = nc.dram_tensor("y", [C, H, WS], mybir.dt.float32, kind="ExternalOutput")

    FD = RPP * WS            # 960 elements per channel per tap
    PFD = C * FD             # 2880 product elements per tap

    with TileContext(nc) as tc:
        with tc.tile_pool(name="const", bufs=1) as const_pool, \
             tc.tile_pool(name="kf32", bufs=4) as kf_pool, \
             tc.tile_pool(name="kbf", bufs=3) as kb_pool, \
             tc.tile_pool(name="prod", bufs=4) as prod_pool, \
             tc.tile_pool(name="out", bufs=1) as out_pool, \
             tc.tile_pool(name="psum", bufs=1, space="PSUM") as psum_pool:

            slab = const_pool.tile([128, 2 * SLABF], mybir.dt.bfloat16)
            ident = const_pool.tile([NP, NP], mybir.dt.bfloat16)
            nc.sync.dma_start(slab[:].rearrange("p (v f) -> p v f", v=2),
                              xs_d.ap().rearrange("v p f -> p v f"))
            nc.sync.dma_start(ident[:], id_d.ap())
            slab_view = slab[:].rearrange(
                "p (v c r w) -> p v c r w", v=2, c=C, r=ROWS_PP, w=WPAD)

            # 6 PSUM accumulators: [channel][half]; halves are flat 512/448 col
            # chunks of the per-channel 960 product columns (1 bank each).
            accs = []
            for c in range(C):
                a0 = psum_pool.tile([NP, 512], mybir.dt.float32, name=f"acc{c}0")
                a1 = psum_pool.tile([NP, 448], mybir.dt.float32, name=f"acc{c}1")
                accs.append((a0, a1))

            npairs = (taps + 1) // 2
            for pi in range(npairs):
                t0 = 2 * pi
                ntap = min(2, taps - t0)
                kf = kf_pool.tile([NP, ntap * FD], mybir.dt.float32, name="kf")
                nc.sync.dma_start(
                    kf[:].rearrange("p (t f) -> p t f", t=ntap),
                    k_d.ap()[t0:t0 + ntap].rearrange("t (p r) w -> p t (r w)", r=RPP))
                kb = kb_pool.tile([NP, ntap * FD], mybir.dt.bfloat16, name="kb")
                nc.scalar.copy(kb[:], kf[:])

                for dt_ in range(ntap):
                    t = t0 + dt_
                    ii, jj = divmod(t, KS)
                    v = jj & 1
                    jj2 = jj - v
                    xs_op = slab_view[0:NP, v, :, ii:ii + RPP, jj2:jj2 + WS]
                    k_op = (kb[0:NP, dt_ * FD:(dt_ + 1) * FD]
                            .rearrange("p (r w) -> p r w", r=RPP)
                            .unsqueeze(1).broadcast_to([NP, C, RPP, WS]))
                    prod = prod_pool.tile([NP, PFD], mybir.dt.bfloat16, name="prod")
                    prod_view = prod[0:NP, :].rearrange(
                        "p (c r w) -> p c r w", c=C, r=RPP, w=WS)
                    nc.vector.tensor_tensor(prod_view, xs_op, k_op,
                                            mybir.AluOpType.mult)
                    first = (t == 0)
                    last = (t == taps - 1)
                    for c in range(C):
                        nc.tensor.matmul(accs[c][0][:], ident[:],
                                         prod[0:NP, c * FD:c * FD + 512],
                                         start=first, stop=last)
                        nc.tensor.matmul(accs[c][1][:], ident[:],
                                         prod[0:NP, c * FD + 512:(c + 1) * FD],
                                         start=first, stop=last)

            yst = out_pool.tile([NP, PFD], mybir.dt.float32)
            for c in range(C):
                nc.scalar.copy(yst[0:NP, c * FD:c * FD + 512], accs[c][0][:])
                nc.scalar.copy(yst[0:NP, c * FD + 512:(c + 1) * FD], accs[c][1][:])
            nc.sync.dma_start(
                y_d.ap().rearrange("c (p r) w -> p c (r w)", r=RPP),
                yst[0:NP, :].rearrange("p (c f) -> p c f", c=C))

    nc.compile()
    return nc


def get_nc(taps=NTAPS):
    if taps not in _CACHE:
        _CACHE[taps] = _build_nc(taps)
    return _CACHE[taps]


def _prep_inputs(x, k, padding, padding_value):
    """Host-side prep: pad x, build bf16 slabs + per-core shards."""
    x = np.asarray(x, dtype=np.float32)
    k = np.asarray(k, dtype=np.float32)
    pad = bool(int(np.asarray(padding)))
    pv = float(np.asarray(padding_value))

    if pad:
        assert x.shape == (1, C, H, W), x.shape
        xp = np.full((C, H + 2 * HALF, W + 2 * HALF), pv, dtype=np.float32)
        xp[:, HALF:HALF + H, HALF:HALF + W] = x[0]
    else:
        assert x.shape == (1, C, H + 2 * HALF, W + 2 * HALF), x.shape
        xp = x[0]

    assert k.shape == (1, NTAPS, H, W), k.shape
    k3 = k[0]

    # row-window index map: partition p holds padded rows [RPP*p, RPP*p+ROWS_PP)
    rows_idx = RPP * np.arange(128)[:, None] + np.arange(ROWS_PP)[None, :]

    ident = np.eye(NP, dtype=BF16)
    in_maps = []
    for ci in range(NCORES):
        w0 = WS * ci
        strip = xp[:, :, w0:w0 + WPAD]                     # [C, 730, 170]
        spad = np.zeros((C, RPP * 127 + ROWS_PP, WPAD + 1), dtype=np.float32)
        spad[:, :H + 2 * HALF, :WPAD] = strip
        xs = np.empty((2, 128, SLABF), dtype=BF16)
        for v in (0, 1):
            sv = spad[:, :, v:v + WPAD]                    # [C, 778, 170]
            win = sv[:, rows_idx, :]                       # [C, 128, 16, 170]
            xs[v] = win.transpose(1, 0, 2, 3).reshape(128, SLABF).astype(BF16)
        kshard = np.ascontiguousarray(k3[:, :, w0:w0 + WS])  # [121, 720, 160]
        in_maps.append({"k": kshard, "xs": xs, "ident": ident})
    return in_maps


def kernel(x, k, padding, padding_value):
    in_maps = _prep_inputs(x, k, padding, padding_value)
    nc = get_nc()
    res = run_bass_kernel_spmd(nc, in_maps, core_ids=list(range(NCORES)))
    y = np.concatenate([res.results[ci]["y"] for ci in range(NCORES)], axis=2)
    return y[None].astype(np.float32)


# revision 3
# speedup vs baseline: 1.1872x; 1.1872x over previous
"""Trainium2 Bass kernel for per-pixel kernel application (KPN-style ApplyKernel).

y[c,h,w] = sum_{ii,jj} xpad[c, h+ii, w+jj] * k[ii*11+jj, h, w]

Strategy (8 NeuronCores, data-parallel over W strips of 160 cols):
  - Host: pad x and build per-partition row-window slabs (6 rows + 10-row halo
    per partition, shifts live in the free dim) in bf16, two column-alignment
    variants so every tap's DVE read stays 4-byte aligned (keeps
    tensor_tensor in its 2x bf16 mode). k is re-laid-out host-side to
    row-block-major [120, 121, 960] so each DMA descriptor moves a large
    contiguous per-partition chunk (42 KB) instead of 3.8 KB.
  - Device, per group of 11 taps: one DMA of the k group (f32->bf16 cast in
    flight on the gpsimd/SWDGE path, or HWDGE + ScalarE cast); per tap one
    VectorE tensor_tensor multiply (bf16 2x mode) and 6 TensorE
    identity-matmuls accumulating into 6 PSUM banks (3 channels x 2 chunks).
  - Epilogue: ScalarE evacuates PSUM -> SBUF, one contiguous DMA out,
    host-side reshape of y.
"""

import sys

if "/opt/trn_rl_repo" not in sys.path:
    sys.path.insert(0, "/opt/trn_rl_repo")

import numpy as np
import ml_dtypes

import concourse.mybir as mybir
from concourse import bacc
from concourse.tile import TileContext
from concourse.bass_utils import run_bass_kernel_spmd

KS = 11
HALF = 5
H, W, C = 720, 1280, 3
NCORES = 8
WS = W // NCORES            # 160 cols per core
RPP = 6                     # output rows per partition
NP = H // RPP               # 120 partitions used
ROWS_PP = RPP + 2 * HALF    # 16 rows stored per partition
WPAD = WS + 2 * HALF        # 170 cols stored per partition
SLABF = C * ROWS_PP * WPAD  # 8160 bf16 per partition per variant
NTAPS = KS * KS             # 121
G = 11                      # taps per k DMA group
NG = NTAPS // G             # 11 groups
FD = RPP * WS               # 960 elements per channel per tap
PFD = C * FD                # 2880 product elements per tap

BF16 = ml_dtypes.bfloat16

_CACHE = {}


def _build_nc(variant="cast", taps=NTAPS):
    nc = bacc.Bacc("TRN2", target_bir_lowering=False, debug=False)
    k_d = nc.dram_tensor("k", [NP, NTAPS, FD], mybir.dt.float32, kind="ExternalInput")
    xs_d = nc.dram_tensor("xs", [2, 128, SLABF], mybir.dt.bfloat16, kind="ExternalInput")
    id_d = nc.dram_tensor("ident", [NP, NP], mybir.dt.bfloat16, kind="ExternalInput")
    y_d = nc.dram_tensor("y", [NP, PFD], mybir.dt.float32, kind="ExternalOutput")

    with TileContext(nc) as tc:
        with tc.tile_pool(name="const", bufs=1) as const_pool, \
             tc.tile_pool(name="kf32", bufs=2) as kf_pool, \
             tc.tile_pool(name="kbf", bufs=2) as kb_pool, \
             tc.tile_pool(name="prod", bufs=4) as prod_pool, \
             tc.tile_pool(name="out", bufs=1) as out_pool, \
             tc.tile_pool(name="psum", bufs=1, space="PSUM") as psum_pool:

            slab = const_pool.tile([128, 2 * SLABF], mybir.dt.bfloat16)
            ident = const_pool.tile([NP, NP], mybir.dt.bfloat16)
            nc.sync.dma_start(slab[:].rearrange("p (v f) -> p v f", v=2),
                              xs_d.ap().rearrange("v p f -> p v f"))
            nc.sync.dma_start(ident[:], id_d.ap())
            slab_view = slab[:].rearrange(
                "p (v c r w) -> p v c r w", v=2, c=C, r=ROWS_PP, w=WPAD)

            accs = []
            for c in range(C):
                a0 = psum_pool.tile([NP, 512], mybir.dt.float32, name=f"acc{c}0")
                a1 = psum_pool.tile([NP, 448], mybir.dt.float32, name=f"acc{c}1")
                accs.append((a0, a1))

            for g in range((taps + G - 1) // G):
                t0 = g * G
                ng = min(G, taps - t0)
                kb = kb_pool.tile([NP, ng * FD], mybir.dt.bfloat16, name="kb")
                ksrc = k_d.ap()[:, t0:t0 + ng, :]
                if variant == "cast":
                    nc.gpsimd.dma_start(
                        kb[:].rearrange("p (t f) -> p t f", t=ng), ksrc)
                else:
                    kf = kf_pool.tile([NP, ng * FD], mybir.dt.float32, name="kf")
                    nc.sync.dma_start(
                        kf[:].rearrange("p (t f) -> p t f", t=ng), ksrc)
                    nc.scalar.copy(kb[:], kf[:])

                for dt_ in range(ng):
                    t = t0 + dt_
                    ii, jj = divmod(t, KS)
                    v = jj & 1
                    jj2 = jj - v
                    xs_op = slab_view[0:NP, v, :, ii:ii + RPP, jj2:jj2 + WS]
                    k_op = (kb[0:NP, dt_ * FD:(dt_ + 1) * FD]
                            .rearrange("p (r w) -> p r w", r=RPP)
                            .unsqueeze(1).broadcast_to([NP, C, RPP, WS]))
                    prod = prod_pool.tile([NP, PFD], mybir.dt.bfloat16, name="prod")
                    prod_view = prod[0:NP, :].rearrange(
                        "p (c r w) -> p c r w", c=C, r=RPP, w=WS)
                    nc.vector.tensor_tensor(prod_view, xs_op, k_op,
                                            mybir.AluOpType.mult)
                    first = (t == 0)
                    last = (t == taps - 1)
                    for c in range(C):
                        nc.tensor.matmul(accs[c][0][:], ident[:],
                                         prod[0:NP, c * FD:c * FD + 512],
                                         start=first, stop=last)
                        nc.tensor.matmul(accs[c][1][:], ident[:],
                                         prod[0:NP, c * FD + 512:(c + 1) * FD],
                                         start=first, stop=last)

            yst = out_pool.tile([NP, PFD], mybir.dt.float32)
            for c in range(C):
                nc.scalar.copy(yst[0:NP, c * FD:c * FD + 512], accs[c][0][:])
                nc.scalar.copy(yst[0:NP, c * FD + 512:(c + 1) * FD], accs[c][1][:])
            nc.sync.dma_start(y_d.ap(), yst[0:NP, :])

    nc.compile()
    return nc


def get_nc(variant="cast", taps=NTAPS):
    key = (variant, taps)
    if key not in _CACHE:
        _CACHE[key] = _build_nc(variant, taps)
    return _CACHE[key]


def _prep_inputs(x, k, padding, padding_value):
    """Host-side prep: pad x, build bf16 slabs + per-core shards."""
    x = np.asarray(x, dtype=np.float32)
    k = np.asarray(k, dtype=np.float32)
    pad = bool(int(np.asarray(padding)))
    pv = float(np.asarray(padding_value))

    if pad:
        assert x.shape == (1, C, H, W), x.shape
        xp = np.full((C, H + 2 * HALF, W + 2 * HALF), pv, dtype=np.float32)
        xp[:, HALF:HALF + H, HALF:HALF + W] = x[0]
    else:
        assert x.shape == (1, C, H + 2 * HALF, W + 2 * HALF), x.shape
        xp = np.ascontiguousarray(x[0])

    assert k.shape == (1, NTAPS, H, W), k.shape
    # row-block-major k: kT[p, t, (r w)] = k[t, RPP*p + r, w]
    kt = np.ascontiguousarray(
        k[0].reshape(NTAPS, NP, RPP * W).transpose(1, 0, 2))  # [120, 121, 6*1280]

    rows_idx = RPP * np.arange(128)[:, None] + np.arange(ROWS_PP)[None, :]
    ident = np.eye(NP, dtype=BF16)
    in_maps = []
    for ci in range(NCORES):
        w0 = WS * ci
        strip = xp[:, :, w0:w0 + WPAD]                     # [C, 730, 170]
        spad = np.zeros((C, RPP * 127 + ROWS_PP, WPAD + 1), dtype=np.float32)
        spad[:, :H + 2 * HALF, :WPAD] = strip
        xs = np.empty((2, 128, SLABF), dtype=BF16)
        for v in (0, 1):
            sv = spad[:, :, v:v + WPAD]                    # [C, 778, 170]
            win = sv[:, rows_idx, :]                       # [C, 128, 16, 170]
            xs[v] = win.transpose(1, 0, 2, 3).reshape(128, SLABF).astype(BF16)
        # k strip in row-block-major: [120, 121, 960]
        kshard = np.ascontiguousarray(
            kt.reshape(NP, NTAPS, RPP, W)[:, :, :, w0:w0 + WS]
            .reshape(NP, NTAPS, FD))
        in_maps.append({"k": kshard, "xs": xs, "ident": ident})
    return in_maps


def _assemble_y(results):
    """results[ci]["y"] is [120, 2880]; reassemble to [1, C, H, W]."""
    y = np.empty((C, H, W), dtype=np.float32)
    for ci in range(NCORES):
        blk = results[ci]["y"].reshape(NP, C, RPP, WS)     # [p, c, r, w]
        y[:, :, WS * ci:WS * (ci + 1)] = blk.transpose(1, 0, 2, 3).reshape(C, H, WS)
    return y[None]


def kernel(x, k, padding, padding_value):
    in_maps = _prep_inputs(x, k, padding, padding_value)
    nc = get_nc()
    res = run_bass_kernel_spmd(nc, in_maps, core_ids=list(range(NCORES)))
    return _assemble_y(res.results).astype(np.float32)
